# revision 24
# baseline (speedup 1.0000x reference)
"""MHA on 8 NeuronCores, v2: query-token-sharded attention.

Core c owns token block c = (batch c//2, seq half c%2), 1024 tokens.
  - Phase 1 (token-parallel): Q^T, K^T, V for my block, all 1024 dims, bf16.
    Q^T never leaves SBUF. K^T and V go to pairwise AllGather (groups
    [2b, 2b+1]) so both cores of a batch hold the batch's full-sequence
    K^T [1024 d, 2048] and V [2048, 1024].
  - Phase 2: dense attention for MY 1024 queries x all 16 heads over the
    batch's 2048 keys. Scores transposed (S^T[k, q]) -> exp on ACT ->
    PV with a ones-row giving the softmax denominator for free; division
    via DRAM-broadcast reciprocal (reshaped [64, 16] so DVE reciprocal is
    cheap). Normalized A^T goes straight into SBUF tiles laid out for the
    output projection.
  - Phase 3: out = A @ wo^T for my tokens, entirely local. Host concatenates
    the 8 disjoint token blocks.

Only communication: 2 pairwise AllGathers (2MB in / 4MB out each),
fully overlapped with phase-1/2 compute. bf16 matmuls, fp32 PSUM.
"""
import numpy as np
import ml_dtypes

import concourse.bass as bass
import concourse.bacc as bacc
import concourse.tile as tile
import concourse.mybir as mybir

N_CORES = 8
P = 128
B, S, D = 4, 2048, 1024
TOK = 1024  # my tokens
CD = D // P
QB = 512
NKC = S // P  # 16 key chunks
F32 = mybir.dt.float32
BF16 = mybir.dt.bfloat16
EXP = mybir.ActivationFunctionType.Exp
PAIR_GROUPS = [[2 * i, 2 * i + 1] for i in range(4)]

_CACHE = {}


def _n_excess_waits(nc):
    import json

    m = json.loads(nc.to_json_bytes())
    insts = [i for f in m["functions"] for b in f["blocks"] for i in b["instructions"]]
    return sum(
        1
        for i in insts
        if len((i.get("sync_info") or {}).get("on_wait", [])) >= 2
        and i.get("opcode") != "EventSemaphore"
    )


def _finish(nc):
    nc.compile()
    import bass_rust

    for _ in range(6):
        if _n_excess_waits(nc) == 0:
            break
        bass_rust.generate_event_semaphores(nc)
    assert _n_excess_waits(nc) == 0, "excess sync waits remain"
    nc.codegen_inst_isa_subclasses()
    return nc


def build_nc(scopes=False):
    nc = bacc.Bacc("TRN2", target_bir_lowering=False, debug=False, num_devices=N_CORES)

    xqT_d = nc.dram_tensor("xqT", [D, TOK], BF16, kind="ExternalInput").ap()
    xkT_d = nc.dram_tensor("xkT", [D, TOK], BF16, kind="ExternalInput").ap()
    xvT_d = nc.dram_tensor("xvT", [D, TOK], BF16, kind="ExternalInput").ap()
    wqkvT = nc.dram_tensor("wqkvT", [D, 3 * D], BF16, kind="ExternalInput").ap()
    woT = nc.dram_tensor("woT", [D, D], BF16, kind="ExternalInput").ap()
    out = nc.dram_tensor("out", [TOK, D], F32, kind="ExternalOutput").ap()

    # pairwise exchange buffers
    kag_i = nc.dram_tensor("kag_i", [D, TOK], BF16).ap()
    kag_oA = nc.dram_tensor("kag_oA", [2, D // 2, TOK], BF16).ap()  # d-chunks 0-3
    kag_oB = nc.dram_tensor("kag_oB", [2, D // 2, TOK], BF16).ap()  # d-chunks 4-7
    vag_i = nc.dram_tensor("vag_i", [TOK, D], BF16).ap()
    vag_os = [
        nc.dram_tensor(f"vag_o{q}", [2, TOK // 4, D], BF16).ap() for q in range(4)
    ]
    den_d = nc.dram_tensor("den_d", [16, TOK], F32).ap()
    recip_d = nc.dram_tensor("recip_d", [16, TOK], F32).ap()

    from contextlib import ExitStack, nullcontext

    def scope(name):
        return nc.named_scope(name) if scopes else nullcontext()

    AG_KW = dict(
        kind="AllGather", op=mybir.AluOpType.bypass, replica_groups=PAIR_GROUPS
    )

    with tile.TileContext(nc) as tc:
        persist = ExitStack()
        qp = persist.enter_context(tc.tile_pool(name="qp", bufs=1))
        wop = persist.enter_context(tc.tile_pool(name="wop", bufs=1))
        ltp = persist.enter_context(tc.tile_pool(name="ltp", bufs=1))

        # ---------------- Phase 1: K, V (exchanged) then Q (stays local) ----
        with ExitStack() as ph1:
            xts = ph1.enter_context(tc.tile_pool(name="xts", bufs=1))
            wp = ph1.enter_context(tc.tile_pool(name="wp", bufs=1))
            ev1 = ph1.enter_context(tc.tile_pool(name="ev1", bufs=4))
            ps1 = ph1.enter_context(tc.tile_pool(name="ps1", bufs=3, space="PSUM"))

            with scope("load"):
                w_t, xqT, xkT, xvT = [], [], [], []
                # interleave w and xk loads so proj_k (first) starts ASAP
                for j in range(CD):
                    wt = wp.tile([P, 3 * D], BF16, name=f"w_{j}")
                    nc.sync.dma_start(out=wt, in_=wqkvT[j * P : (j + 1) * P, :])
                    w_t.append(wt)
                    t = xts.tile([P, TOK], BF16, name=f"xkT_{j}")
                    nc.sync.dma_start(out=t, in_=xkT_d[j * P : (j + 1) * P, :])
                    xkT.append(t)
                for nm, x, lst in (("v", xvT_d, xvT), ("q", xqT_d, xqT)):
                    for j in range(CD):
                        t = xts.tile([P, TOK], BF16, name=f"x{nm}T_{j}")
                        nc.sync.dma_start(out=t, in_=x[j * P : (j + 1) * P, :])
                        lst.append(t)

            # K^T [d-chunk, tok] -> kag_i
            with scope("proj_k"):
                for i in range(CD):
                    ps = ps1.tile([P, TOK], F32, name="ps_k", tag="ps1")
                    for j in range(CD):
                        lhsT = w_t[j][:, D + i * P : D + (i + 1) * P]
                        for h in range(TOK // QB):
                            nc.tensor.matmul(
                                ps[:, h * QB : (h + 1) * QB],
                                lhsT,
                                xkT[j][:, h * QB : (h + 1) * QB],
                                start=(j == 0),
                                stop=(j == CD - 1),
                            )
                    sb = ev1.tile([P, TOK], BF16, name="sb_k", tag="ev1")
                    (nc.scalar.copy if i % 2 == 0 else nc.vector.tensor_copy)(sb, ps)
                    nc.sync.dma_start(out=kag_i[i * P : (i + 1) * P, :], in_=sb)
            with scope("ag_k"):
                nc.gpsimd.collective_compute(
                    ins=[kag_i[0 : D // 2, :]], outs=[kag_oA[:]], **AG_KW
                )
                nc.gpsimd.collective_compute(
                    ins=[kag_i[D // 2 : D, :]], outs=[kag_oB[:]], **AG_KW
                )

            # Q^T [d-chunk, tok] -> SBUF (persistent)
            with scope("proj_q"):
                qT_t = []
                for i in range(CD):
                    ps = ps1.tile([P, TOK], F32, name="ps_q", tag="ps1")
                    for j in range(CD):
                        lhsT = w_t[j][:, i * P : (i + 1) * P]
                        for h in range(TOK // QB):
                            nc.tensor.matmul(
                                ps[:, h * QB : (h + 1) * QB],
                                lhsT,
                                xqT[j][:, h * QB : (h + 1) * QB],
                                start=(j == 0),
                                stop=(j == CD - 1),
                            )
                    qt = qp.tile([P, TOK], BF16, name=f"qT_{i}")
                    (nc.scalar.copy if i % 2 == 0 else nc.vector.tensor_copy)(qt, ps)
                    qT_t.append(qt)

            # V [tok-chunk, d] -> vag_i
            with scope("proj_v"):
                for t_i in range(CD):
                    ps = ps1.tile([P, D], F32, name="ps_v", tag="ps1")
                    for j in range(CD):
                        lhsT = xvT[j][:, t_i * P : (t_i + 1) * P]
                        for h in range(D // QB):
                            nc.tensor.matmul(
                                ps[:, h * QB : (h + 1) * QB],
                                lhsT,
                                w_t[j][:, 2 * D + h * QB : 2 * D + (h + 1) * QB],
                                start=(j == 0),
                                stop=(j == CD - 1),
                            )
                    sb = ev1.tile([P, D], BF16, name="sb_v", tag="ev1")
                    (nc.scalar.copy if t_i % 2 == 0 else nc.vector.tensor_copy)(sb, ps)
                    nc.sync.dma_start(out=vag_i[t_i * P : (t_i + 1) * P, :], in_=sb)
            with scope("ag_v"):
                for q in range(4):
                    nc.gpsimd.collective_compute(
                        ins=[vag_i[q * TOK // 4 : (q + 1) * TOK // 4, :]],
                        outs=[vag_os[q][:]],
                        **AG_KW,
                    )

        # ---------------- Phase 2: attention, 16 heads x my 1024 queries ----
        with ExitStack() as ph2:
            kst = ph2.enter_context(tc.tile_pool(name="kst", bufs=1))
            vp = ph2.enter_context(tc.tile_pool(name="vp", bufs=3))
            pt = ph2.enter_context(tc.tile_pool(name="pt", bufs=4))
            at = ph2.enter_context(tc.tile_pool(name="at", bufs=3))
            sm = ph2.enter_context(tc.tile_pool(name="sm", bufs=2))
            ps2 = ExitStack()
            s_ps = ps2.enter_context(tc.tile_pool(name="s_ps", bufs=2, space="PSUM"))
            pv_ps = ps2.enter_context(tc.tile_pool(name="pv_ps", bufs=2, space="PSUM"))

            # woT prefetch (phase 3) and lt output tiles
            wo_t = []
            for j in range(CD):
                wt3 = wop.tile([P, D], BF16, name=f"wo_{j}")
                nc.sync.dma_start(out=wt3, in_=woT[j * P : (j + 1) * P, :])
                wo_t.append(wt3)
            lts = [ltp.tile([P, TOK], BF16, name=f"lt_{i}") for i in range(CD)]

            # stage gathered K^T as 8 SBUF tiles [128 d-chunk, 2048 k]
            kT_s = []
            for j in range(CD):
                t = kst.tile([P, S], BF16, name=f"kTs_{j}")
                kg = kag_oA if j < 4 else kag_oB
                jj = j % 4
                nc.sync.dma_start(out=t[:, 0:TOK], in_=kg[0, jj * P : (jj + 1) * P, :])
                nc.sync.dma_start(out=t[:, TOK:S], in_=kg[1, jj * P : (jj + 1) * P, :])
                kT_s.append(t)

            KCS = [0, 1, 8, 9, 2, 3, 10, 11, 4, 5, 12, 13, 6, 7, 14, 15]
            GROUPS = [(0, 3), (3, 6), (6, 9), (9, 12), (12, 15), (15, 16)]

            # flatten (head, qblock, group) into a software pipeline with a
            # one-group scores lookahead so ACT(exp) never waits on PE latency
            units = []  # (h, qb) state
            vts, araws, pvs = {}, {}, {}

            def load_head(h):
                v_t = vp.tile([P, NKC, 65], BF16, name="v_t", tag="vp")
                for q in range(4):
                    for half in range(2):
                        vsrc = vag_os[q][half, :, 64 * h : 64 * h + 64]
                        nc.sync.dma_start(
                            out=v_t[:, 4 * q + 2 * half : 4 * q + 2 * half + 2, 0:64],
                            in_=vsrc.rearrange("(kc p) d -> p kc d", p=P),
                        )
                nc.vector.memset(v_t[:, :, 64:65], 1.0)
                vts[h] = v_t

            steps = [
                (h, qb, gi)
                for h in range(16)
                for qb in range(TOK // QB)
                for gi in range(len(GROUPS))
            ]

            def emit_scores(step):
                h, qb, gi = step
                if qb == 0 and gi == 0:
                    load_head(h)
                    araws[h] = at.tile([65, TOK], F32, name="a_raw", tag="at")
                g0, g1 = GROUPS[gi]
                if gi == 0:
                    pvs[(h, qb)] = pv_ps.tile([65, QB], F32, name="pv", tag="pv_ps")
                r = slice(64 * (h % 2), 64 * (h % 2) + 64)
                qs = slice(qb * QB, (qb + 1) * QB)
                sg = s_ps.tile([P, 3, QB], F32, name="sg", tag="s_ps")
                for pos in range(g0, g1):
                    kc = KCS[pos]
                    nc.tensor.matmul(
                        sg[:, pos - g0, :],
                        kT_s[h // 2][r, kc * P : (kc + 1) * P],
                        qT_t[h // 2][r, qs],
                        start=True,
                        stop=True,
                    )
                return sg

            def emit_exp_pv(step, sg):
                h, qb, gi = step
                g0, g1 = GROUPS[gi]
                n = g1 - g0
                pg = pt.tile([P, 3, QB], BF16, name="pg", tag="pt")
                nc.scalar.activation(pg[:, 0:n, :], sg[:, 0:n, :], EXP, scale=0.125)
                return pg

            def emit_pv(step, pg):
                h, qb, gi = step
                g0, g1 = GROUPS[gi]
                for pos in range(g0, g1):
                    nc.tensor.matmul(
                        pvs[(h, qb)],
                        vts[h][:, pos, :],
                        pg[:, pos - g0, :],
                        start=(pos == 0),
                        stop=(pos == NKC - 1),
                    )
                if g1 == NKC:
                    qs = slice(qb * QB, (qb + 1) * QB)
                    nc.vector.tensor_copy(araws[h][:, qs], pvs[(h, qb)])
                    if qb == TOK // QB - 1:
                        finish_head(h)

            def finish_head(h):
                with scope(f"norm_h{h}"):
                    a_raw = araws.pop(h)
                    nc.sync.dma_start(out=den_d[h : h + 1, :], in_=a_raw[64:65, :])
                    dsq = sm.tile([64, 16], F32, name="dsq", tag="smd")
                    nc.sync.dma_start(
                        out=dsq,
                        in_=bass.AP(
                            tensor=den_d.tensor,
                            offset=h * TOK,
                            ap=[[16, 64], [1, 16]],
                        ),
                    )
                    rsq = sm.tile([64, 16], F32, name="rsq", tag="smr")
                    nc.vector.reciprocal(rsq, dsq)
                    nc.sync.dma_start(
                        out=bass.AP(
                            tensor=recip_d.tensor,
                            offset=h * TOK,
                            ap=[[16, 64], [1, 16]],
                        ),
                        in_=rsq,
                    )
                    bc = at.tile([64, TOK], F32, name="bc", tag="at2")
                    nc.sync.dma_start(
                        out=bc,
                        in_=bass.AP(
                            tensor=recip_d.tensor,
                            offset=h * TOK,
                            ap=[[0, 64], [1, TOK]],
                        ),
                    )
                    rr = slice(64 * (h % 2), 64 * (h % 2) + 64)
                    nc.vector.tensor_mul(lts[h // 2][rr, :], a_raw[0:64, :], bc)

            with scope("attn"):
                sg_cur = emit_scores(steps[0])
                for i, step in enumerate(steps):
                    pg = emit_exp_pv(step, sg_cur)
                    if i + 1 < len(steps):
                        sg_cur = emit_scores(steps[i + 1])
                    emit_pv(step, pg)

            ps2.close()

            # ---------------- Phase 3: output projection (local) -------------
            with scope("wo"):
                ev3 = ph2.enter_context(tc.tile_pool(name="ev3", bufs=3))
                ps3p = ph2.enter_context(tc.tile_pool(name="ps3p", bufs=3, space="PSUM"))
                for t_i in range(CD):
                    ps3 = ps3p.tile([P, D], F32, name="ps3", tag="ps3")
                    for sc in range(CD):
                        for hh in range(2):
                            nc.tensor.matmul(
                                ps3[:, hh * QB : (hh + 1) * QB],
                                lts[sc][:, t_i * P : (t_i + 1) * P],
                                wo_t[sc][:, hh * QB : (hh + 1) * QB],
                                start=(sc == 0),
                                stop=(sc == CD - 1),
                            )
                    ob = ev3.tile([P, D], F32, name="ob", tag="ev3")
                    nc.vector.tensor_copy(ob, ps3)
                    nc.sync.dma_start(out=out[t_i * P : (t_i + 1) * P, :], in_=ob)

        persist.close()

    return _finish(nc)


def _get_nc(scopes=False):
    key = ("nc", scopes)
    if key not in _CACHE:
        _CACHE[key] = build_nc(scopes)
    return _CACHE[key]


def make_in_maps(query, key, value, wq, wk, wv, wo):
    qf = np.asarray(query, np.float32).reshape(B * S, D)
    kf = np.asarray(key, np.float32).reshape(B * S, D)
    vf = np.asarray(value, np.float32).reshape(B * S, D)
    wqkvT = np.ascontiguousarray(
        np.concatenate([np.asarray(wq), np.asarray(wk), np.asarray(wv)], 0).T
    ).astype(ml_dtypes.bfloat16)
    woT_h = np.ascontiguousarray(np.asarray(wo).T).astype(ml_dtypes.bfloat16)
    in_maps = []
    for c in range(N_CORES):
        sl = slice(c * TOK, (c + 1) * TOK)
        in_maps.append(
            {
                "xqT": np.ascontiguousarray(qf[sl].T).astype(ml_dtypes.bfloat16),
                "xkT": np.ascontiguousarray(kf[sl].T).astype(ml_dtypes.bfloat16),
                "xvT": np.ascontiguousarray(vf[sl].T).astype(ml_dtypes.bfloat16),
                "wqkvT": wqkvT,
                "woT": woT_h,
            }
        )
    return in_maps


def assemble(results):
    blocks = [results[c]["out"] for c in range(N_CORES)]
    return np.concatenate(blocks, 0).reshape(B, S, D).astype(np.float32)


def kernel(query, key, value, mask, wq, wk, wv, wo):
    # mask is all-False in this problem: softmax without masking.
    nc = _get_nc()
    in_maps = make_in_maps(query, key, value, wq, wk, wv, wo)
    from concourse.bass_utils import run_bass_kernel_spmd

    res = run_bass_kernel_spmd(nc, in_maps, list(range(N_CORES)))
    return assemble(res.results)
